# revision 4
# baseline (speedup 1.0000x reference)
"""Trainium2 Bass kernel: batched time-domain cross-correlation.

For each of 2048 (=64x32) independent pairs (fp32):
    out[g, l] = sum_k d1[g, k + l - 301] * d2[g, k],   l in [0, 603)

Algorithm: overlap-save block correlation in the half-shift (negacyclic)
real-DFT basis of length N = 2B (B = 384), with the x-window transforms
computed from per-block HALF-transforms (radix-2 reuse):

  window w_s = [xb_s; xb_{s+1}]  (hop B, length 2B)
  P_b = F_lo^T xb_b   (F_lo = first B rows of the [N, 2B] basis)
  X_s[k]      = P_s[k]      + (-1)^k P_{s+1}[B+k]     (Re half)
  X_s[B+k]    = P_s[B+k]    - (-1)^k P_{s+1}[k]       (Im half)
  Y_v = F_lo^T yb_v  (y blocks are zero-padded, so F_lo suffices)
  Z_c[k] = sum_v X_{v+c} conj(Y_v)   (c in {0,1})
  out[384c + j] = (Minv Z_c)[j]

All matmuls in bf16 (full PE rate); the pointwise products and the
v-sum (binary add-tree) run on DVE as scalar_tensor_tensor ops, which
hit the 4x bf16 DVE perf mode. PSUM->SBUF spectra copies are large
multi-bank Activation-engine copies (a couple offloaded to GpSimd).

Sharding: data-parallel over the 2048 pairs, 256 pairs per core.
"""

import sys
import time

import numpy as np

if "/opt/trn_rl_repo" not in sys.path:
    sys.path.insert(0, "/opt/trn_rl_repo")

import concourse.bacc as bacc
import concourse.bass as bass
import concourse.mybir as mybir
import concourse.tile as tile
from concourse.bass_utils import run_bass_kernel_spmd

import ml_dtypes

BF16 = ml_dtypes.bfloat16

# ---- problem constants ----
NB_PAIRS, NCH, NT = 64, 32, 3000
LAGS = 603
SHIFT = 301
NCORES = 8
G = (NB_PAIRS * NCH) // NCORES  # 256 pairs per core

# ---- algorithm constants ----
B = 384          # hop / block size
N = 2 * B        # transform length
V = 8            # y blocks  (V*B = 3072 >= 3000)
C = 2            # output lag blocks (C*B = 768 >= 603)
S = V + C - 1    # x windows
NBX = S + 1      # x blocks (10; NBX*B = 3840 >= 3602)
NRG = 6          # 128-bin groups (2B/128): 0..2 Re, 3..5 Im
NQ = 3           # contraction chunks per block (B/128)
GH = 128         # pairs per g-chunk
NCHUNK = G // GH  # 2
NM = 5           # output lag groups of 128 (640 >= 603)

DT = mybir.dt.bfloat16
F32 = mybir.dt.float32

_PE_CACHE = {}
LAST_EXEC_NS = None
LAST_TRACE = None


def _matrices():
    n = np.arange(N, dtype=np.float64)[:, None]
    k = np.arange(B, dtype=np.float64)[None, :]
    theta = np.pi * n * (2 * k + 1) / N
    ffull = np.concatenate([np.cos(theta), -np.sin(theta)], axis=1)  # [N, 2B]
    minv = np.linalg.inv(ffull.T)[:B, :]  # [B, 2B]
    return ffull, minv


def _const_tiles():
    ffull, minv = _matrices()
    flo = ffull[:B, :]  # [B, 2B]
    # fw[p, r, q, j] = F_lo[128q + p, 128r + j]
    fw = np.zeros((128, NRG, NQ, 128), dtype=np.float64)
    for r in range(NRG):
        for q in range(NQ):
            fw[:, r, q, :] = flo[128 * q : 128 * (q + 1), 128 * r : 128 * (r + 1)]
    # ei[p, m, r, j] = minv[jlag0(m) + j, 128r + p]
    ei = np.zeros((128, NM, NRG, 128), dtype=np.float64)
    for m in range(NM):
        j0 = 128 * m if m < 3 else 128 * (m - 3)
        for r in range(NRG):
            blk = minv[j0 : j0 + 128, 128 * r : 128 * (r + 1)]  # [j, p]
            ei[:, m, r, :] = blk.T
    sk = np.zeros((128, 2), dtype=np.float32)
    sk[:, 0] = (-1.0) ** np.arange(128)
    sk[:, 1] = -sk[:, 0]
    return fw.astype(BF16), ei.astype(BF16), sk


def build_kernel():
    nc = bacc.Bacc(
        "TRN2",
        target_bir_lowering=False,
        debug=False,
        num_devices=NCORES,
    )

    xq_d = nc.dram_tensor("xq", [128, NBX, G, NQ], DT, kind="ExternalInput")
    yq_d = nc.dram_tensor("yq", [128, V, G, NQ], DT, kind="ExternalInput")
    fw_d = nc.dram_tensor("fw", [128, NRG, NQ, 128], DT, kind="ExternalInput")
    ei_d = nc.dram_tensor("ei", [128, NM, NRG, 128], DT, kind="ExternalInput")
    sk_d = nc.dram_tensor("sk", [128, 2], F32, kind="ExternalInput")
    out_d = nc.dram_tensor("out", [128, NM, G], F32, kind="ExternalOutput")

    ALUT = mybir.AluOpType

    with tile.TileContext(nc, trace_sim=False) as tc:
        with (
            tc.tile_pool(name="const", bufs=1) as cpool,
            tc.tile_pool(name="io", bufs=2) as iopool,
            tc.tile_pool(name="spec", bufs=1) as spool,
            tc.tile_pool(name="work", bufs=1) as wpool,
            tc.tile_pool(name="psum", bufs=1, space=bass.MemorySpace.PSUM) as ppool,
        ):
            fw_t = cpool.tile([128, NRG, NQ, 128], DT, tag="fw")
            ei_t = cpool.tile([128, NM, NRG, 128], DT, tag="ei")
            sk_t = cpool.tile([128, 2], F32, tag="sk")
            nc.sync.dma_start(fw_t[:], fw_d.ap())
            nc.sync.dma_start(ei_t[:], ei_d.ap())
            nc.sync.dma_start(sk_t[:], sk_d.ap())

            outt = cpool.tile([128, NM, G], F32, tag="outt")

            for ch in range(NCHUNK):
                g0 = ch * GH
                xin = iopool.tile([128, NBX, GH, NQ], DT, tag="xin", bufs=2)
                yin = iopool.tile([128, V, GH, NQ], DT, tag="yin", bufs=2)
                nc.sync.dma_start(xin[:], xq_d.ap()[:, :, g0 : g0 + GH, :])
                nc.sync.dma_start(yin[:], yq_d.ap()[:, :, g0 : g0 + GH, :])

                Ps = spool.tile([128, NRG, NBX, GH], DT, tag="Ps")
                Ys = spool.tile([128, NRG, V, GH], DT, tag="Ys")
                Xs = spool.tile([128, NRG, S, GH], DT, tag="Xs")

                # ---- x half-block forward transforms ----
                for r in range(NRG):
                    pP = ppool.tile([128, 1280], F32, tag="pP", bufs=1)
                    for lo, hi in ((0, 4), (4, 8), (8, 10)):
                        o0, o1 = lo * 128, hi * 128
                        for q in range(NQ):
                            nc.tensor.matmul(
                                pP[:, o0:o1],
                                fw_t[:, r, q, :],
                                xin[:, lo:hi, :, q],
                                start=(q == 0),
                                stop=(q == NQ - 1),
                            )
                    with nc.allow_low_precision("spectra copy to bf16"):
                        if r < 5:
                            nc.scalar.copy(out=Ps[:, r, :, :], in_=pP[:])
                        else:
                            nc.vector.tensor_copy(out=Ps[:, r, :, :], in_=pP[:])

                # ---- y forward transforms (pairs of bin groups) ----
                for rr in range(NRG // 2):
                    pY = ppool.tile([128, 2, V, GH], F32, tag="pY", bufs=1)
                    for h in range(2):
                        r = 2 * rr + h
                        for lo, hi in ((0, 4), (4, 8)):
                            for q in range(NQ):
                                nc.tensor.matmul(
                                    pY[:, h, lo:hi, :],
                                    fw_t[:, r, q, :],
                                    yin[:, lo:hi, :, q],
                                    start=(q == 0),
                                    stop=(q == NQ - 1),
                                )
                    with nc.allow_low_precision("spectra copy to bf16"):
                        nc.scalar.copy(out=Ys[:, 2 * rr : 2 * rr + 2, :, :], in_=pY[:])

                # ---- window combine: X_s from P_s, P_{s+1} ----
                with nc.allow_low_precision("bf16 combine"):
                    nc.vector.scalar_tensor_tensor(
                        out=Xs[:, 0:3, :, :],
                        in0=Ps[:, 3:6, 1 : S + 1, :],
                        scalar=sk_t[:, 0:1],
                        in1=Ps[:, 0:3, 0:S, :],
                        op0=ALUT.mult,
                        op1=ALUT.add,
                    )
                    nc.vector.scalar_tensor_tensor(
                        out=Xs[:, 3:6, :, :],
                        in0=Ps[:, 0:3, 1 : S + 1, :],
                        scalar=sk_t[:, 1:2],
                        in1=Ps[:, 3:6, 0:S, :],
                        op0=ALUT.mult,
                        op1=ALUT.add,
                    )

                # ---- pointwise products + v-sum trees ----
                PP = wpool.tile([128, NRG, V, 2, GH], DT, tag="PP")
                T1 = wpool.tile([128, NRG, 4, 2, GH], DT, tag="T1")
                T2 = wpool.tile([128, NRG, 2, 2, GH], DT, tag="T2")
                T4 = wpool.tile([128, NRG, 2, GH], DT, tag="T4")
                Z = spool.tile([128, NRG, 2, GH], DT, tag="Z")

                for path in range(2):  # 0: Zr (rr+ii), 1: Zi (ir-ri)
                    with nc.allow_low_precision("bf16 products"):
                        for c in range(C):
                            if path == 0:
                                nc.vector.scalar_tensor_tensor(
                                    out=PP[:, :, :, c, :],
                                    in0=Xs[:, :, c : c + V, :],
                                    scalar=1.0,
                                    in1=Ys[:, :, :, :],
                                    op0=ALUT.mult,
                                    op1=ALUT.mult,
                                )
                            else:
                                nc.vector.scalar_tensor_tensor(
                                    out=PP[:, 0:3, :, c, :],
                                    in0=Xs[:, 3:6, c : c + V, :],
                                    scalar=1.0,
                                    in1=Ys[:, 0:3, :, :],
                                    op0=ALUT.mult,
                                    op1=ALUT.mult,
                                )
                                nc.vector.scalar_tensor_tensor(
                                    out=PP[:, 3:6, :, c, :],
                                    in0=Xs[:, 0:3, c : c + V, :],
                                    scalar=-1.0,
                                    in1=Ys[:, 3:6, :, :],
                                    op0=ALUT.mult,
                                    op1=ALUT.mult,
                                )
                        # v-sum binary tree (contiguous halves keep 4x mode)
                        nc.vector.scalar_tensor_tensor(
                            out=T1[:], in0=PP[:, :, 0:4, :, :], scalar=1.0,
                            in1=PP[:, :, 4:8, :, :], op0=ALUT.mult, op1=ALUT.add,
                        )
                        nc.vector.scalar_tensor_tensor(
                            out=T2[:], in0=T1[:, :, 0:2, :, :], scalar=1.0,
                            in1=T1[:, :, 2:4, :, :], op0=ALUT.mult, op1=ALUT.add,
                        )
                        nc.vector.scalar_tensor_tensor(
                            out=T4[:], in0=T2[:, :, 0, :, :], scalar=1.0,
                            in1=T2[:, :, 1, :, :], op0=ALUT.mult, op1=ALUT.add,
                        )
                        nc.vector.scalar_tensor_tensor(
                            out=Z[:, 3 * path : 3 * path + 3, :, :],
                            in0=T4[:, 0:3, :, :], scalar=1.0,
                            in1=T4[:, 3:6, :, :], op0=ALUT.mult, op1=ALUT.add,
                        )

                # ---- inverse transform + lag extraction ----
                pZ = ppool.tile([128, 1280], F32, tag="pP", bufs=1)
                for m in range(NM):
                    c = 0 if m < 3 else 1
                    for r in range(NRG):
                        nc.tensor.matmul(
                            pZ[:, 128 * m : 128 * (m + 1)],
                            ei_t[:, m, r, :],
                            Z[:, r, c, :],
                            start=(r == 0),
                            stop=(r == NRG - 1),
                        )
                nc.scalar.copy(
                    out=outt[:, :, g0 : g0 + GH],
                    in_=pZ[:, 0 : 128 * NM],
                )

            nc.sync.dma_start(out_d.ap()[:], outt[:])

    nc.compile()
    return nc


def _prep_core_inputs(d1f, d2f, fw, ei, sk, core):
    """d1f/d2f: [2048, 3000] fp32. Returns the in_map for `core`."""
    sl = slice(core * G, (core + 1) * G)
    x = d1f[sl]
    y = d2f[sl]
    xp = np.zeros((G, NBX * B), dtype=np.float32)
    xp[:, SHIFT : SHIFT + NT] = x
    yp = np.zeros((G, V * B), dtype=np.float32)
    yp[:, :NT] = y
    # xq[p, b, g, q] = xp[g, 384b + 128q + p]
    xq = np.ascontiguousarray(
        xp.reshape(G, NBX, NQ, 128).transpose(3, 1, 0, 2)
    ).astype(BF16)
    yq = np.ascontiguousarray(
        yp.reshape(G, V, NQ, 128).transpose(3, 1, 0, 2)
    ).astype(BF16)
    return {"xq": xq, "yq": yq, "fw": fw, "ei": ei, "sk": sk}


def kernel(data1: np.ndarray, data2: np.ndarray) -> np.ndarray:
    d1f = np.ascontiguousarray(data1, dtype=np.float32).reshape(-1, NT)
    d2f = np.ascontiguousarray(data2, dtype=np.float32).reshape(-1, NT)
    fw, ei, sk = _const_tiles()

    t0 = time.time()
    if "nc" not in _PE_CACHE:
        _PE_CACHE["nc"] = build_kernel()
    nc = _PE_CACHE["nc"]
    print(f"[kernel] build+compile {time.time() - t0:.1f}s", file=sys.stderr,
          flush=True)

    in_maps = [_prep_core_inputs(d1f, d2f, fw, ei, sk, i) for i in range(NCORES)]
    t0 = time.time()
    res = run_bass_kernel_spmd(nc, in_maps, core_ids=list(range(NCORES)))
    print(f"[kernel] spmd run {time.time() - t0:.1f}s", file=sys.stderr, flush=True)
    global LAST_EXEC_NS, LAST_TRACE
    LAST_EXEC_NS = res.exec_time_ns
    LAST_TRACE = res.instructions_and_trace
    if res.exec_time_ns is not None:
        print(f"[kernel] HW exec {res.exec_time_ns} ns", file=sys.stderr, flush=True)

    outs = []
    for i in range(NCORES):
        o = res.results[i]["out"]  # [128, NM, G] fp32
        # lag(p, m) = 128m + p for m<3 (c=0); 384 + 128(m-3) + p for m>=3
        full = o.transpose(2, 1, 0).reshape(G, NM * 128)  # [g, m*128+p]
        res_g = np.empty((G, LAGS), dtype=np.float32)
        res_g[:, 0:384] = full[:, 0:384]
        res_g[:, 384:603] = full[:, 384 : 384 + 219]
        outs.append(res_g)
    return np.concatenate(outs, axis=0).reshape(NB_PAIRS, NCH, LAGS)


# revision 5
# speedup vs baseline: 1.3625x; 1.3625x over previous
"""Trainium2 Bass kernel: batched time-domain cross-correlation.

For each of 2048 (=64x32) independent pairs (fp32):
    out[g, l] = sum_k d1[g, k + l - 301] * d2[g, k],   l in [0, 603)

Algorithm: overlap-save block correlation in the half-shift (negacyclic)
real-DFT basis of length N = 2B (B = 384), with the x-window transforms
computed from per-block HALF-transforms (radix-2 reuse):

  window w_s = [xb_s; xb_{s+1}]  (hop B, length 2B)
  P_b = F_lo^T xb_b   (F_lo = first B rows of the [N, 2B] basis)
  X_s[k]      = P_s[k]      + (-1)^k P_{s+1}[B+k]     (Re half)
  X_s[B+k]    = P_s[B+k]    - (-1)^k P_{s+1}[k]       (Im half)
  Y_v = F_lo^T yb_v  (y blocks are zero-padded, so F_lo suffices)
  Z_c[k] = sum_v X_{v+c} conj(Y_v)   (c in {0,1})
  out[384c + j] = (Minv Z_c)[j]

x block 9 ([3456,3840)) lies past the padded support [301,3301) and is
identically zero, so only 9 x half-transforms are computed and window 8
is a plain copy of P_8.

All matmuls in bf16 (full PE rate); pointwise products and the v-sum
binary add-tree run on DVE as 2-byte tensor_tensor ops (2x DVE mode),
the (-1)^k combine uses tensor_scalar (4x mode). PSUM->SBUF spectra
copies are large multi-bank Activation-engine copies; a slice of the
product/tree work is offloaded to GpSimd to balance engines.

Sharding: data-parallel over the 2048 pairs, 256 pairs per core.
"""

import sys
import time

import numpy as np

if "/opt/trn_rl_repo" not in sys.path:
    sys.path.insert(0, "/opt/trn_rl_repo")

import concourse.bacc as bacc
import concourse.bass as bass
import concourse.mybir as mybir
import concourse.tile as tile
from concourse.bass_utils import run_bass_kernel_spmd

import ml_dtypes

BF16 = ml_dtypes.bfloat16

# ---- problem constants ----
NB_PAIRS, NCH, NT = 64, 32, 3000
LAGS = 603
SHIFT = 301
NCORES = 8
G = (NB_PAIRS * NCH) // NCORES  # 256 pairs per core

# ---- algorithm constants ----
B = 384          # hop / block size
N = 2 * B        # transform length
V = 8            # y blocks  (V*B = 3072 >= 3000)
C = 2            # output lag blocks (C*B = 768 >= 603)
S = V + C - 1    # x windows (9)
NBX = 9          # nonzero x blocks (block 9 is all-zero padding)
NRG = 6          # 128-bin groups (2B/128): 0..2 Re, 3..5 Im
NQ = 3           # contraction chunks per block (B/128)
GH = 128         # pairs per g-chunk
NCHUNK = G // GH  # 2
NM = 5           # output lag groups of 128 (640 >= 603)

DT = mybir.dt.bfloat16
F32 = mybir.dt.float32

_PE_CACHE = {}
LAST_EXEC_NS = None
LAST_TRACE = None


def _matrices():
    n = np.arange(N, dtype=np.float64)[:, None]
    k = np.arange(B, dtype=np.float64)[None, :]
    theta = np.pi * n * (2 * k + 1) / N
    ffull = np.concatenate([np.cos(theta), -np.sin(theta)], axis=1)  # [N, 2B]
    minv = np.linalg.inv(ffull.T)[:B, :]  # [B, 2B]
    return ffull, minv


def _const_tiles():
    ffull, minv = _matrices()
    flo = ffull[:B, :]  # [B, 2B]
    # fw[p, r, q, j] = F_lo[128q + p, 128r + j]
    fw = np.zeros((128, NRG, NQ, 128), dtype=np.float64)
    for r in range(NRG):
        for q in range(NQ):
            fw[:, r, q, :] = flo[128 * q : 128 * (q + 1), 128 * r : 128 * (r + 1)]
    # ei[p, m, r, j] = minv[jlag0(m) + j, 128r + p]
    ei = np.zeros((128, NM, NRG, 128), dtype=np.float64)
    for m in range(NM):
        j0 = 128 * m if m < 3 else 128 * (m - 3)
        for r in range(NRG):
            blk = minv[j0 : j0 + 128, 128 * r : 128 * (r + 1)]  # [j, p]
            ei[:, m, r, :] = blk.T
    sk = np.zeros((128, 2), dtype=np.float32)
    sk[:, 0] = (-1.0) ** np.arange(128)
    sk[:, 1] = -sk[:, 0]
    return fw.astype(BF16), ei.astype(BF16), sk


def build_kernel():
    nc = bacc.Bacc(
        "TRN2",
        target_bir_lowering=False,
        debug=False,
        num_devices=NCORES,
    )

    xq_d = nc.dram_tensor("xq", [128, NBX, G, NQ], DT, kind="ExternalInput")
    yq_d = nc.dram_tensor("yq", [128, V, G, NQ], DT, kind="ExternalInput")
    fw_d = nc.dram_tensor("fw", [128, NRG, NQ, 128], DT, kind="ExternalInput")
    ei_d = nc.dram_tensor("ei", [128, NM, NRG, 128], DT, kind="ExternalInput")
    sk_d = nc.dram_tensor("sk", [128, 2], F32, kind="ExternalInput")
    out_d = nc.dram_tensor("out", [128, NM, G], F32, kind="ExternalOutput")

    ALUT = mybir.AluOpType

    with tile.TileContext(nc, trace_sim=False) as tc:
        with (
            tc.tile_pool(name="const", bufs=1) as cpool,
            tc.tile_pool(name="io", bufs=2) as iopool,
            tc.tile_pool(name="spec", bufs=1) as spool,
            tc.tile_pool(name="work", bufs=1) as wpool,
            tc.tile_pool(name="psum", bufs=1, space=bass.MemorySpace.PSUM) as ppool,
        ):
            fw_t = cpool.tile([128, NRG, NQ, 128], DT, tag="fw")
            ei_t = cpool.tile([128, NM, NRG, 128], DT, tag="ei")
            sk_t = cpool.tile([128, 2], F32, tag="sk")
            nc.sync.dma_start(fw_t[:], fw_d.ap())
            nc.sync.dma_start(ei_t[:], ei_d.ap())
            nc.sync.dma_start(sk_t[:], sk_d.ap())

            outt = cpool.tile([128, NM, G], F32, tag="outt")

            for ch in range(NCHUNK):
                g0 = ch * GH
                xin = iopool.tile([128, NBX, GH, NQ], DT, tag="xin", bufs=2)
                yin = iopool.tile([128, V, GH, NQ], DT, tag="yin", bufs=2)
                nc.sync.dma_start(xin[:], xq_d.ap()[:, :, g0 : g0 + GH, :])
                nc.sync.dma_start(yin[:], yq_d.ap()[:, :, g0 : g0 + GH, :])

                Ps = spool.tile([128, NRG, NBX, GH], DT, tag="Ps")
                Ys = spool.tile([128, NRG, V, GH], DT, tag="Ys")
                Xs = spool.tile([128, NRG, S, GH], DT, tag="Xs")
                Xt = spool.tile([128, NRG, S - 1, GH], DT, tag="Xt")

                # ---- x half-block forward transforms (9 blocks) ----
                for r in range(NRG):
                    pP = ppool.tile([128, NBX * GH], F32, tag="pP", bufs=1)
                    for lo, hi in ((0, 4), (4, 8), (8, 9)):
                        for q in range(NQ):
                            nc.tensor.matmul(
                                pP[:, lo * GH : hi * GH],
                                fw_t[:, r, q, :],
                                xin[:, lo:hi, :, q],
                                start=(q == 0),
                                stop=(q == NQ - 1),
                            )
                    with nc.allow_low_precision("spectra copy to bf16"):
                        nc.scalar.copy(out=Ps[:, r, :, :], in_=pP[:])

                # ---- y forward transforms (pairs of bin groups) ----
                for rr in range(NRG // 2):
                    pY = ppool.tile([128, 2, V, GH], F32, tag="pY", bufs=1)
                    for h in range(2):
                        r = 2 * rr + h
                        for lo, hi in ((0, 4), (4, 8)):
                            for q in range(NQ):
                                nc.tensor.matmul(
                                    pY[:, h, lo:hi, :],
                                    fw_t[:, r, q, :],
                                    yin[:, lo:hi, :, q],
                                    start=(q == 0),
                                    stop=(q == NQ - 1),
                                )
                    with nc.allow_low_precision("spectra copy to bf16"):
                        nc.scalar.copy(out=Ys[:, 2 * rr : 2 * rr + 2, :, :], in_=pY[:])

                # ---- window combine: X_s = P_s + (-1)^k * swap(P_{s+1}) ----
                with nc.allow_low_precision("bf16 combine"):
                    nc.vector.tensor_scalar_mul(
                        Xt[:, 0:3, :, :], Ps[:, 3:6, 1:NBX, :], sk_t[:, 0:1]
                    )
                    nc.vector.tensor_scalar_mul(
                        Xt[:, 3:6, :, :], Ps[:, 0:3, 1:NBX, :], sk_t[:, 1:2]
                    )
                    nc.vector.tensor_add(
                        Xs[:, :, 0 : S - 1, :], Ps[:, :, 0 : S - 1, :], Xt[:]
                    )
                    # window S-1 pairs with the all-zero block 9
                    nc.vector.tensor_copy(
                        out=Xs[:, :, S - 1, :], in_=Ps[:, :, S - 1, :]
                    )

                # ---- pointwise products + v-sum trees ----
                PP = wpool.tile([128, NRG, 2, V, GH], DT, tag="PP")
                T1 = wpool.tile([128, NRG, 2, 4, GH], DT, tag="T1")
                T2 = wpool.tile([128, NRG, 2, 2, GH], DT, tag="T2")
                T4 = wpool.tile([128, NRG, 2, GH], DT, tag="T4")
                Z = spool.tile([128, NRG, 2, GH], DT, tag="Z")

                for path in range(2):  # 0: Zr (rr+ii), 1: Zi (ir, ri)
                    with nc.allow_low_precision("bf16 products"):
                        for c in range(C):
                            if path == 0:
                                nc.vector.tensor_mul(
                                    PP[:, :, c, :, :],
                                    Xs[:, :, c : c + V, :],
                                    Ys[:, :, :, :],
                                )
                            else:
                                nc.vector.tensor_mul(
                                    PP[:, 0:3, c, :, :],
                                    Xs[:, 3:6, c : c + V, :],
                                    Ys[:, 0:3, :, :],
                                )
                                nc.gpsimd.tensor_mul(
                                    PP[:, 3:6, c, :, :],
                                    Xs[:, 0:3, c : c + V, :],
                                    Ys[:, 3:6, :, :],
                                )
                        # v-sum binary tree (contiguous halves keep 2x mode)
                        nc.vector.tensor_add(
                            T1[:], PP[:, :, :, 0:4, :], PP[:, :, :, 4:8, :]
                        )
                        nc.vector.tensor_add(
                            T2[:], T1[:, :, :, 0:2, :], T1[:, :, :, 2:4, :]
                        )
                        eng = nc.gpsimd if path == 1 else nc.vector
                        eng.tensor_add(
                            T4[:], T2[:, :, :, 0, :], T2[:, :, :, 1, :]
                        )
                        # group-pair combine; conj sign for the Zi path
                        if path == 0:
                            nc.gpsimd.tensor_add(
                                Z[:, 0:3, :, :], T4[:, 0:3, :, :], T4[:, 3:6, :, :]
                            )
                        else:
                            nc.gpsimd.tensor_sub(
                                Z[:, 3:6, :, :], T4[:, 0:3, :, :], T4[:, 3:6, :, :]
                            )

                # ---- inverse transform + lag extraction ----
                pZ = ppool.tile([128, NBX * GH], F32, tag="pP", bufs=1)
                for m in range(NM):
                    c = 0 if m < 3 else 1
                    for r in range(NRG):
                        nc.tensor.matmul(
                            pZ[:, GH * m : GH * (m + 1)],
                            ei_t[:, m, r, :],
                            Z[:, r, c, :],
                            start=(r == 0),
                            stop=(r == NRG - 1),
                        )
                nc.scalar.copy(
                    out=outt[:, :, g0 : g0 + GH],
                    in_=pZ[:, 0 : GH * NM],
                )

            nc.sync.dma_start(out_d.ap()[:], outt[:])

    nc.compile()
    return nc


def _prep_core_inputs(d1f, d2f, fw, ei, sk, core):
    """d1f/d2f: [2048, 3000] fp32. Returns the in_map for `core`."""
    sl = slice(core * G, (core + 1) * G)
    x = d1f[sl]
    y = d2f[sl]
    xp = np.zeros((G, NBX * B), dtype=np.float32)
    xp[:, SHIFT : SHIFT + NT] = x
    yp = np.zeros((G, V * B), dtype=np.float32)
    yp[:, :NT] = y
    # xq[p, b, g, q] = xp[g, 384b + 128q + p]
    xq = np.ascontiguousarray(
        xp.reshape(G, NBX, NQ, 128).transpose(3, 1, 0, 2)
    ).astype(BF16)
    yq = np.ascontiguousarray(
        yp.reshape(G, V, NQ, 128).transpose(3, 1, 0, 2)
    ).astype(BF16)
    return {"xq": xq, "yq": yq, "fw": fw, "ei": ei, "sk": sk}


def kernel(data1: np.ndarray, data2: np.ndarray) -> np.ndarray:
    d1f = np.ascontiguousarray(data1, dtype=np.float32).reshape(-1, NT)
    d2f = np.ascontiguousarray(data2, dtype=np.float32).reshape(-1, NT)
    fw, ei, sk = _const_tiles()

    t0 = time.time()
    if "nc" not in _PE_CACHE:
        _PE_CACHE["nc"] = build_kernel()
    nc = _PE_CACHE["nc"]
    print(f"[kernel] build+compile {time.time() - t0:.1f}s", file=sys.stderr,
          flush=True)

    in_maps = [_prep_core_inputs(d1f, d2f, fw, ei, sk, i) for i in range(NCORES)]
    t0 = time.time()
    res = run_bass_kernel_spmd(nc, in_maps, core_ids=list(range(NCORES)))
    print(f"[kernel] spmd run {time.time() - t0:.1f}s", file=sys.stderr, flush=True)
    global LAST_EXEC_NS, LAST_TRACE
    LAST_EXEC_NS = res.exec_time_ns
    LAST_TRACE = res.instructions_and_trace
    if res.exec_time_ns is not None:
        print(f"[kernel] HW exec {res.exec_time_ns} ns", file=sys.stderr, flush=True)

    outs = []
    for i in range(NCORES):
        o = res.results[i]["out"]  # [128, NM, G] fp32
        # lag(p, m) = 128m + p for m<3 (c=0); 384 + 128(m-3) + p for m>=3
        full = o.transpose(2, 1, 0).reshape(G, NM * 128)  # [g, m*128+p]
        res_g = np.empty((G, LAGS), dtype=np.float32)
        res_g[:, 0:384] = full[:, 0:384]
        res_g[:, 384:603] = full[:, 384 : 384 + 219]
        outs.append(res_g)
    return np.concatenate(outs, axis=0).reshape(NB_PAIRS, NCH, LAGS)
